# revision 4
# baseline (speedup 1.0000x reference)
"""Trainium2 Bass kernel for batched weighted scatter-add (AttentionCopy).

Computes out[b, o, v] = sum_i attn[b, o, i] * (ids[b, i] == v)
for ids [16, 512] int32 in [0, 50000), attn [16, 32, 512] f32,
out [16, 32, 50000] f32.

Pure data parallel over batch: 2 batches per core on 8 cores. Per batch the
[32, 50000] output is built densely as 7 SBUF tiles: 6 of [128, 2048] f32
(vocab span 8192 = 4 groups x 2048; seed-0 data puts at most 98 of 512 ids
in any such span, capacity 128) plus a [128, 212] tail. Tile rows are
(o, g) pairs (p = o*4 + g, g = rel // 2048), columns lo = rel % 2048, so
each partition's 8 KB row is contiguous in DRAM and one descriptor moves
8 KB (vs 2.5 KB in the v1 4096-span kernel - per-SDMA-engine rate improves
from ~24 to ~26 GB/s).

Per tile the device does a one-hot matmul pass: alo[i, c] = (lo_i == c)
(built from a DMA-loaded iota constant in two halves, c and c-1024, since
fp16 is integer-exact only to 2048), then out[(o,g), c] = gt.T @ alo with
gt[i, (o,g)] = attn[b, o, i] masked by (g == hi_i), packed on host.

Two hardware pathologies dominate the v1 profile and are addressed here:

1. SDMA engine 15 runs at ~17.8 GB/s vs ~24 for engines 0-14 (known trn2
   quirk), and partition->engine mapping is fixed (port = bits[4:2]<<1 |
   bit[6]; partitions {92-95, 124-127} -> engine 15). With a uniform AP it
   carries 1/16 of the 12.8 MB/core output and sets the critical path. Fix:
   those partitions (rows o=23, o=31) only DMA columns [0:SPLIT]; the
   remaining columns are re-emitted by a second small matmul (stationary =
   the same 8 gt columns, zero-padded into a [128,128] buffer at a
   per-tile rotating partition block) so their data lands on partitions
   served by other engines, and are written from there.

2. The PE HAM clock gate holds the tensor engine at 1.2 GHz unless it is
   nearly always busy (observed: matmuls at 630 ns vs the 216 ns warm rate,
   oscillating K=4/8 most of the run). Dummy "warmer" matmuls into a scratch
   PSUM bank pad each tile's PE work so the array stays at K=8/8.

The v1 gpsimd iota (1.8 us, serializing the first alo build) is replaced by
a DMA-loaded [128, 1024] fp16 iota input; gpsimd instead builds the alo
one-hots and the spill stationary (its SBUF port is otherwise idle), leaving
scalar = psA copy + half the DMA kicks, vector = psB + spill copies, sync =
the other DMA kicks, all below the ~2.5 us/tile DMA cadence.
"""

import sys

sys.path.insert(0, "/opt/trn_rl_repo")

import numpy as np

NCORES = 8
B, O, I = 16, 32, 512
SIZE = 50000
BPC = B // NCORES  # batches per core
V2 = 2048  # per-partition columns per full tile (fp16 int-exact limit)
V2T = 212  # tail tile: 6*4*2048 + 4*212 = 50000
GPT = 4  # groups per tile: 128 rows = 32 o x 4 groups
SPAN = GPT * V2  # 8192 vocab per full tile
TILES = 7  # 6 full + 1 tail per batch
KW = 128  # id-window capacity per (batch, tile)
NW = BPC * TILES  # 14 windows per core
NFULL = BPC * (TILES - 1)  # 12 full windows per core
LA = 2  # alo build lookahead (windows)
SPLIT = 1232  # engine-15 partitions write cols [0:SPLIT] directly
SPILL = V2 - SPLIT  # 816 cols re-routed via the spill matmul
NWARM = 2  # PE warmer matmuls per full window (HAM K=8/8 hold)
# per-window rotation of the 8-partition spill block (each block feeds two
# SDMA engines; blocks containing engine-15 or engine-13 partitions excluded)
ROT = [0, 64, 8, 72, 16, 80, 24, 96, 32, 104, 40, 112]
# port-15 rows: p = o*4+g for o in {23, 31}
O15A, O15B = 23, 31

_cache = {}


def _tile_v2(t):
    return V2 if t < TILES - 1 else V2T


def _tile_off(t):
    return t * SPAN  # tail starts at 6*8192 = 49152


def _build():
    import concourse.bacc as bacc
    import concourse.mybir as mybir
    import concourse.tile as tile

    f32 = mybir.dt.float32
    f16 = mybir.dt.float16
    Alu = mybir.AluOpType

    nc = bacc.Bacc("TRN2", target_bir_lowering=False, debug=False, num_devices=NCORES)

    # host-packed stationary matrices: [b, i_slot, t*KW + (o*4+g)]
    gt_d = nc.dram_tensor("gtj", [BPC, 128, TILES * KW], f16, kind="ExternalInput").ap()
    # spill stationary columns: [b, i_slot, t*8 + j], j in 0..7 <-> (o in
    # {23,31}) x (g in 0..3)
    g8_d = nc.dram_tensor("g8j", [BPC, 128, (TILES - 1) * 8], f16,
                          kind="ExternalInput").ap()
    # lo and lo-1024 per window slot: [p, 2*(b*TILES+t)] (pad: 0 / -1024)
    lof_d = nc.dram_tensor("lof", [128, 2 * NW], f32, kind="ExternalInput").ap()
    # iota constant lov[p, c] = c
    lov_d = nc.dram_tensor("lov", [128, 1024], f16, kind="ExternalInput").ap()
    out_d = nc.dram_tensor("out", [BPC, O, SIZE], f32, kind="ExternalOutput").ap()

    with tile.TileContext(nc) as tc:
        with (
            tc.tile_pool(name="const", bufs=1) as constp,
            tc.tile_pool(name="alo", bufs=LA + 2) as alop,
            tc.tile_pool(name="stat", bufs=LA + 2) as statp,
            tc.tile_pool(name="outs", bufs=4) as outp,
            tc.tile_pool(name="spls", bufs=2) as splsp,
            tc.tile_pool(name="psmm", bufs=2, space="PSUM") as psmm,
            tc.tile_pool(name="psspill", bufs=1, space="PSUM") as psspill,
            tc.tile_pool(name="pswarm", bufs=1, space="PSUM") as pswarm,
        ):
            # --- inputs: iota constant first (gates the first alo build),
            # then tail gt blocks (first windows), then the rest
            lov = constp.tile([128, 1024], f16, tag="lov")
            nc.sync.dma_start(out=lov[:, 0:512], in_=lov_d[:, 0:512])
            nc.scalar.dma_start(out=lov[:, 512:1024], in_=lov_d[:, 512:1024])

            lo_f = constp.tile([128, 2 * NW], f32, tag="lo_f")
            nc.scalar.dma_start(out=lo_f[:], in_=lof_d[:])

            TKW = TILES * KW
            tb0, tb1 = (TILES - 1) * KW, TILES * KW  # tail gt column block
            gts, g8s = [], []
            for b in range(BPC):
                t_ = constp.tile([128, TKW], f16, tag=f"gt{b}", name=f"gt{b}")
                g_ = constp.tile([128, (TILES - 1) * 8], f16, tag=f"g8{b}",
                                 name=f"g8{b}")
                nc.sync.dma_start(out=t_[:, tb0:tb1], in_=gt_d[b][:, tb0:tb1])
                nc.sync.dma_start(out=t_[:, 0:384], in_=gt_d[b][:, 0:384])
                nc.scalar.dma_start(out=g_[:], in_=g8_d[b][:])
                nc.scalar.dma_start(out=t_[:, 384:tb0], in_=gt_d[b][:, 384:tb0])
                gts.append(t_)
                g8s.append(g_)

            # warmer scratch: one PSUM bank, WAW-chained matmuls keep the PE
            # array busy enough that HAM holds the 2.4 GHz clock
            wps = pswarm.tile([128, 512], f32, tag="warm")

            # window order: tails lead each batch (their small-descriptor DMA
            # hides in the ramp instead of serializing the drain)
            order = [(0, TILES - 1)] + [(0, t) for t in range(TILES - 1)]
            order += [(1, TILES - 1)] + [(1, t) for t in range(TILES - 1)]

            alos = {}
            stats = {}
            spill_serial = {}
            s = 0
            for (b, t) in order:
                if t < TILES - 1:
                    spill_serial[(b, t)] = s
                    s += 1

            def build(w):
                b, t = w
                v2 = _tile_v2(t)
                wi = b * TILES + t
                alo = alop.tile([128, V2], f16, tag="alo", name=f"alo{b}_{t}")
                if v2 == V2:
                    nc.gpsimd.tensor_scalar(
                        out=alo[:, 0:1024], in0=lov[:, 0:1024],
                        scalar1=lo_f[:, 2 * wi : 2 * wi + 1],
                        scalar2=None, op0=Alu.is_equal)
                    nc.gpsimd.tensor_scalar(
                        out=alo[:, 1024:2048], in0=lov[:, 0:1024],
                        scalar1=lo_f[:, 2 * wi + 1 : 2 * wi + 2],
                        scalar2=None, op0=Alu.is_equal)
                    # spill stationary: gt8 block scattered into a zeroed
                    # [128, 128] buffer at this window's rotation offset
                    r = ROT[spill_serial[w]]
                    st = statp.tile([128, 128], f16, tag="stat",
                                    name=f"st{b}_{t}")
                    nc.gpsimd.memset(st[:], 0)
                    nc.gpsimd.tensor_copy(
                        out=st[:, r : r + 8],
                        in_=g8s[b][:, t * 8 : (t + 1) * 8])
                    stats[w] = (st, r)
                else:
                    nc.gpsimd.tensor_scalar(
                        out=alo[:, 0:V2T], in0=lov[:, 0:V2T],
                        scalar1=lo_f[:, 2 * wi : 2 * wi + 1],
                        scalar2=None, op0=Alu.is_equal)
                alos[w] = alo

            for k in range(LA):
                build(order[k])

            for k in range(len(order)):
                if k + LA < len(order):
                    build(order[k + LA])
                b, t = w = order[k]
                v2 = _tile_v2(t)
                alo = alos[w]
                gt = gts[b][:, t * KW : (t + 1) * KW]
                qa, qb = ((nc.sync, nc.scalar) if k % 2 == 0
                          else (nc.scalar, nc.sync))

                off = _tile_off(t)
                outv = out_d[b][:, off : off + GPT * v2].rearrange(
                    "o (g l) -> o g l", l=v2)

                if t == TILES - 1:
                    # tail: one matmul, scalar copy, single DMA
                    ps = psmm.tile([128, 1024], f32, tag="mm", name=f"ps{b}t")
                    nc.tensor.matmul(out=ps[:, 0:V2T], lhsT=gt,
                                     rhs=alo[:, 0:V2T], start=True, stop=True)
                    os_ = outp.tile([128, V2], f32, tag="os", name=f"os{b}t")
                    nc.scalar.copy(out=os_[:, 0:V2T], in_=ps[:, 0:V2T])
                    qa.dma_start(out=outv, in_=os_[:, 0:V2T])
                    continue

                st, r = stats[w]
                psA = psmm.tile([128, 1024], f32, tag="mm", name=f"psA{b}_{t}")
                psB = psmm.tile([128, 1024], f32, tag="mm", name=f"psB{b}_{t}")
                psS = psspill.tile([128, SPILL], f32, tag="sp",
                                   name=f"psS{b}_{t}")
                nc.tensor.matmul(out=psA[:, 0:512], lhsT=gt,
                                 rhs=alo[:, 0:512], start=True, stop=True)
                nc.tensor.matmul(out=psA[:, 512:1024], lhsT=gt,
                                 rhs=alo[:, 512:1024], start=True, stop=True)
                nc.tensor.matmul(out=psB[:, 0:512], lhsT=gt,
                                 rhs=alo[:, 1024:1536], start=True, stop=True)
                nc.tensor.matmul(out=psB[:, 512:1024], lhsT=gt,
                                 rhs=alo[:, 1536:2048], start=True, stop=True)
                nc.tensor.matmul(out=psS[:, 0:512], lhsT=st[:, 0:128],
                                 rhs=alo[:, SPLIT : SPLIT + 512],
                                 start=True, stop=True)
                nc.tensor.matmul(out=psS[:, 512:SPILL], lhsT=st[:, 0:128],
                                 rhs=alo[:, SPLIT + 512 : V2],
                                 start=True, stop=True)

                os_ = outp.tile([128, V2], f32, tag="os", name=f"os{b}_{t}")
                os2 = splsp.tile([128, SPILL], f32, tag="os2",
                                 name=f"os2{b}_{t}")
                nc.scalar.copy(out=os_[:, 0:1024], in_=psA[:, 0:1024])
                nc.vector.tensor_copy(out=os_[:, 1024:2048],
                                      in_=psB[:, 0:1024])
                # full-partition copy (DVE PSUM reads need aligned partition
                # bases; lanes are parallel so width-816 cost is identical)
                nc.vector.tensor_copy(out=os2[:, 0:SPILL],
                                      in_=psS[:, 0:SPILL])

                # PE warmers (scratch bank; no consumers)
                for _ in range(NWARM):
                    nc.tensor.matmul(out=wps[:, 0:512], lhsT=lov[:, 0:128],
                                     rhs=lov[:, 0:512], start=True, stop=True)

                if k == 1:
                    # first full window: kick each column half as soon as its
                    # copy lands to start the output stream earlier
                    qa.dma_start(out=outv[0:O15A, :, 0:1024],
                                 in_=os_[0 : 4 * O15A, 0:1024])
                    qb.dma_start(out=outv[O15A + 1 : O15B, :, 0:1024],
                                 in_=os_[4 * O15A + 4 : 4 * O15B, 0:1024])
                    qa.dma_start(out=outv[0:O15A, :, 1024:2048],
                                 in_=os_[0 : 4 * O15A, 1024:2048])
                    qb.dma_start(out=outv[O15A + 1 : O15B, :, 1024:2048],
                                 in_=os_[4 * O15A + 4 : 4 * O15B, 1024:2048])
                else:
                    qa.dma_start(out=outv[0:O15A],
                                 in_=os_[0 : 4 * O15A, 0:2048])
                    qb.dma_start(out=outv[O15A + 1 : O15B],
                                 in_=os_[4 * O15A + 4 : 4 * O15B, 0:2048])
                # engine-15 rows, direct part
                qb.dma_start(out=outv[O15A : O15A + 1, :, 0:SPLIT],
                             in_=os_[4 * O15A : 4 * O15A + 4, 0:SPLIT])
                qb.dma_start(out=outv[O15B : O15B + 1, :, 0:SPLIT],
                             in_=os_[4 * O15B : 4 * O15B + 4, 0:SPLIT])
                # spill part from the rotated partitions
                qa.dma_start(out=outv[O15A : O15A + 1, :, SPLIT:V2],
                             in_=os2[r : r + 4, 0:SPILL])
                qa.dma_start(out=outv[O15B : O15B + 1, :, SPLIT:V2],
                             in_=os2[r + 4 : r + 8, 0:SPILL])

    nc.compile()
    return nc


def _in_maps(ids, attn):
    lo_w = np.zeros((B, TILES, 2, KW), dtype=np.float32)
    lo_w[:, :, 1, :] = -1024.0
    gt_w = np.zeros((B, TILES, KW, KW), dtype=np.float16)
    g8_w = np.zeros((B, TILES - 1, KW, 8), dtype=np.float16)
    oi = np.arange(O)
    o8 = np.array([O15A] * 4 + [O15B] * 4)
    g8 = np.array([0, 1, 2, 3, 0, 1, 2, 3])
    for b in range(B):
        for t in range(TILES):
            off, v2 = _tile_off(t), _tile_v2(t)
            sel = np.nonzero((ids[b] >= off) & (ids[b] < off + GPT * v2))[0]
            c = sel.size
            if c > KW:
                raise RuntimeError(
                    f"id window overflow: batch {b} span {t} has {c} > {KW} ids"
                )
            rel = ids[b, sel] - off
            hi = rel // v2
            lo = rel % v2
            lo_w[b, t, 0, :c] = lo
            lo_w[b, t, 1, :c] = lo - 1024.0
            # gt[i, o*4+g] = attn[b, o, sel[i]] * (hi[i] == g)
            cols = attn[b][:, sel].T.astype(np.float16)  # [c, O]
            gt_w[b, t, np.arange(c)[:, None], oi[None, :] * GPT + hi[:, None]] = cols
            if t < TILES - 1:
                # g8[i, j] = attn[b, o8[j], sel[i]] * (hi[i] == g8[j])
                m = hi[:, None] == g8[None, :]
                g8_w[b, t, :c, :] = np.where(
                    m, attn[b][o8, :][:, sel].T.astype(np.float16), 0)
    # lof: [8 cores, 128 slots, 2*NW]
    lo_t = lo_w.reshape(NCORES, NW * 2, KW).transpose(0, 2, 1)
    gt_t = gt_w.reshape(NCORES, BPC, TILES, KW, KW).transpose(
        0, 1, 3, 2, 4).reshape(NCORES, BPC, KW, TILES * KW)
    g8_t = g8_w.reshape(NCORES, BPC, TILES - 1, KW, 8).transpose(
        0, 1, 3, 2, 4).reshape(NCORES, BPC, KW, (TILES - 1) * 8)
    lov = np.broadcast_to(
        np.arange(1024, dtype=np.float16)[None, :], (128, 1024))
    in_maps = [
        {
            "gtj": np.ascontiguousarray(gt_t[c]),
            "g8j": np.ascontiguousarray(g8_t[c]),
            "lof": np.ascontiguousarray(lo_t[c]),
            "lov": np.ascontiguousarray(lov),
        }
        for c in range(NCORES)
    ]
    return in_maps


def kernel(ids, attn):
    from concourse.bass_utils import run_bass_kernel_spmd

    ids = np.ascontiguousarray(ids, dtype=np.int32)
    attn = np.ascontiguousarray(attn, dtype=np.float32)

    if "nc" not in _cache:
        _cache["nc"] = _build()
    nc = _cache["nc"]

    core_ids = list(range(NCORES))
    res = run_bass_kernel_spmd(nc, _in_maps(ids, attn), core_ids)
    out = np.concatenate([res.results[c]["out"] for c in core_ids], axis=0)
    return out


# revision 5
# speedup vs baseline: 7.7997x; 7.7997x over previous
"""Trainium2 Bass kernel for batched weighted scatter-add (AttentionCopy).

Computes out[b, o, v] = sum_i attn[b, o, i] * (ids[b, i] == v)
for ids [16, 512] int32 in [0, 50000), attn [16, 32, 512] f32,
out [16, 32, 50000] f32.

Pure data parallel over batch: 2 batches per core on 8 cores. Per batch the
[32, 50000] output is built densely as 7 SBUF tiles: 6 of [128, 2048] f32
(vocab span 8192 = 4 groups x 2048; seed-0 data puts at most 98 of 512 ids
in any such span, capacity 128) plus a [128, 212] tail. Tile rows are
(o, g) pairs (p = o*4 + g, g = rel // 2048), columns lo = rel % 2048, so
each partition's 8 KB row is contiguous in DRAM: descriptors are 8 KB
(vs 2.5 KB in the v1 4096-span kernel; measured per-SDMA-engine rate
improves ~24 -> ~25.5 GB/s) and there are 4x fewer of them.

Each output DMA is a [32, 4, 2048] access pattern. The HWDGE sprays
descriptors over the 16 SDMA engines BY OUTER-DIM INDEX (engine = o mod
16) - NOT by SBUF partition - and only outer counts that are <= 16 or
divisible by 16 spray at all (a [23, 4, 2048] AP lands entirely on engine
0, measured). So outer dim stays 32 and each engine moves one [4, 2048]
block for its two o values.

Per tile the device does a one-hot matmul pass: alo[i, c] = (lo_i == c)
(single vector op per tile; fp16 holds integers exactly up to 2048), then
out[(o,g), c] = gt.T @ alo with gt[i, (o,g)] = attn[b, o, i] masked by
(g == hi_i), packed on host. The compare constant lov[p, c] = c is a DMA
input (the v1 gpsimd iota took 1.8 us and serialized the first build; the
gpsimd tensor_scalar path is a ~40x-slow emulation - measured - so all
builds stay on the vector engine).

The PE HAM clock gate holds the tensor engine at 1.2 GHz unless it is
nearly always busy (v1: 630 ns matmuls vs the 216 ns warm rate). Dummy
"warmer" matmuls into a scratch PSUM bank pad each tile's PE work so the
array stays at K=8/8.

Steady-state per-tile engine budget (target cadence ~2.6 us = 1 MB tile /
16 engines / 25.5 GB/s): vector = alo build 0.6 + psB copy 1.2; scalar =
psA copy 1.0 + every-other-tile DMA kick 0.45; tensor = 4 matmuls + 2
warmers ~1.9 warm; sync = other DMA kicks.
"""

import sys

sys.path.insert(0, "/opt/trn_rl_repo")

import numpy as np

NCORES = 8
B, O, I = 16, 32, 512
SIZE = 50000
BPC = B // NCORES  # batches per core
V2 = 2048  # per-partition columns per full tile (fp16 int-exact limit)
V2T = 212  # tail tile: 6*4*2048 + 4*212 = 50000
GPT = 4  # groups per tile: 128 rows = 32 o x 4 groups
SPAN = GPT * V2  # 8192 vocab per full tile
TILES = 7  # 6 full + 1 tail per batch
KW = 128  # id-window capacity per (batch, tile)
NW = BPC * TILES  # 14 windows per core
LA = 2  # alo build lookahead (windows)
NWARM = 2  # PE warmer matmuls per full window (HAM K=8/8 hold)

_cache = {}


def _tile_v2(t):
    return V2 if t < TILES - 1 else V2T


def _tile_off(t):
    return t * SPAN  # tail starts at 6*8192 = 49152


def _build():
    import concourse.bacc as bacc
    import concourse.mybir as mybir
    import concourse.tile as tile

    f32 = mybir.dt.float32
    f16 = mybir.dt.float16
    Alu = mybir.AluOpType

    nc = bacc.Bacc("TRN2", target_bir_lowering=False, debug=False, num_devices=NCORES)

    # host-packed stationary matrices: [b, i_slot, t*KW + (o*4+g)]
    gt_d = nc.dram_tensor("gtj", [BPC, 128, TILES * KW], f16, kind="ExternalInput").ap()
    # lo per window slot: [p, b*TILES+t] (0 pad)
    lof_d = nc.dram_tensor("lof", [128, NW], f32, kind="ExternalInput").ap()
    # iota constant lov[p, c] = c
    lov_d = nc.dram_tensor("lov", [128, V2], f16, kind="ExternalInput").ap()
    out_d = nc.dram_tensor("out", [BPC, O, SIZE], f32, kind="ExternalOutput").ap()

    with tile.TileContext(nc) as tc:
        with (
            tc.tile_pool(name="const", bufs=1) as constp,
            tc.tile_pool(name="alo", bufs=LA + 2) as alop,
            tc.tile_pool(name="outs", bufs=4) as outp,
            tc.tile_pool(name="psmm", bufs=3, space="PSUM") as psmm,
            tc.tile_pool(name="pswarm", bufs=1, space="PSUM") as pswarm,
        ):
            # --- inputs: iota constant first (gates the first alo build),
            # then tail gt blocks (first windows), then the rest
            lov = constp.tile([128, V2], f16, tag="lov")
            nc.sync.dma_start(out=lov[:, 0:1024], in_=lov_d[:, 0:1024])
            nc.scalar.dma_start(out=lov[:, 1024:2048], in_=lov_d[:, 1024:2048])

            lo_f = constp.tile([128, NW], f32, tag="lo_f")
            nc.scalar.dma_start(out=lo_f[:], in_=lof_d[:])

            TKW = TILES * KW
            tb0, tb1 = (TILES - 1) * KW, TILES * KW  # tail gt column block
            gts = []
            for b in range(BPC):
                t_ = constp.tile([128, TKW], f16, tag=f"gt{b}", name=f"gt{b}")
                nc.sync.dma_start(out=t_[:, tb0:tb1], in_=gt_d[b][:, tb0:tb1])
                nc.sync.dma_start(out=t_[:, 0:384], in_=gt_d[b][:, 0:384])
                nc.scalar.dma_start(out=t_[:, 384:tb0], in_=gt_d[b][:, 384:tb0])
                gts.append(t_)

            # warmer scratch: one PSUM bank, WAW-chained matmuls keep the PE
            # array busy enough that HAM holds the 2.4 GHz clock
            wps = pswarm.tile([128, 512], f32, tag="warm")

            # window order: tails lead each batch (their small-descriptor DMA
            # hides in the ramp instead of serializing the drain)
            order = [(0, TILES - 1)] + [(0, t) for t in range(TILES - 1)]
            order += [(1, TILES - 1)] + [(1, t) for t in range(TILES - 1)]

            alos = {}

            def build(w):
                b, t = w
                v2 = _tile_v2(t)
                wi = b * TILES + t
                alo = alop.tile([128, V2], f16, tag="alo", name=f"alo{b}_{t}")
                nc.vector.tensor_scalar(
                    out=alo[:, 0:v2], in0=lov[:, 0:v2],
                    scalar1=lo_f[:, wi : wi + 1],
                    scalar2=None, op0=Alu.is_equal)
                alos[w] = alo

            for k in range(LA):
                build(order[k])

            for k in range(len(order)):
                if k + LA < len(order):
                    build(order[k + LA])
                b, t = w = order[k]
                v2 = _tile_v2(t)
                alo = alos[w]
                gt = gts[b][:, t * KW : (t + 1) * KW]
                qa = nc.sync if k % 2 == 0 else nc.scalar

                off = _tile_off(t)
                outv = out_d[b][:, off : off + GPT * v2].rearrange(
                    "o (g l) -> o g l", l=v2)

                if t == TILES - 1:
                    # tail: one matmul, scalar copy, single DMA
                    ps = psmm.tile([128, 1024], f32, tag="mm", name=f"ps{b}t")
                    nc.tensor.matmul(out=ps[:, 0:V2T], lhsT=gt,
                                     rhs=alo[:, 0:V2T], start=True, stop=True)
                    os_ = outp.tile([128, V2], f32, tag="os", name=f"os{b}t")
                    nc.scalar.copy(out=os_[:, 0:V2T], in_=ps[:, 0:V2T])
                    qa.dma_start(out=outv, in_=os_[:, 0:V2T])
                    continue

                psA = psmm.tile([128, 1024], f32, tag="mm", name=f"psA{b}_{t}")
                psB = psmm.tile([128, 1024], f32, tag="mm", name=f"psB{b}_{t}")
                nc.tensor.matmul(out=psA[:, 0:512], lhsT=gt,
                                 rhs=alo[:, 0:512], start=True, stop=True)
                nc.tensor.matmul(out=psA[:, 512:1024], lhsT=gt,
                                 rhs=alo[:, 512:1024], start=True, stop=True)
                nc.tensor.matmul(out=psB[:, 0:512], lhsT=gt,
                                 rhs=alo[:, 1024:1536], start=True, stop=True)
                nc.tensor.matmul(out=psB[:, 512:1024], lhsT=gt,
                                 rhs=alo[:, 1536:2048], start=True, stop=True)

                os_ = outp.tile([128, V2], f32, tag="os", name=f"os{b}_{t}")
                nc.scalar.copy(out=os_[:, 0:1024], in_=psA[:, 0:1024])
                nc.vector.tensor_copy(out=os_[:, 1024:2048],
                                      in_=psB[:, 0:1024])

                # PE warmers (scratch bank; no consumers)
                for _ in range(NWARM):
                    nc.tensor.matmul(out=wps[:, 0:512], lhsT=lov[:, 0:128],
                                     rhs=lov[:, 0:512], start=True, stop=True)

                if k == 1:
                    # first full window: kick each column half as soon as its
                    # copy lands to start the output stream earlier
                    qa.dma_start(out=outv[:, :, 0:1024],
                                 in_=os_[:, 0:1024])
                    qa.dma_start(out=outv[:, :, 1024:2048],
                                 in_=os_[:, 1024:2048])
                else:
                    qa.dma_start(out=outv, in_=os_[:, 0:2048])

    nc.compile()
    return nc


def _in_maps(ids, attn):
    lo_w = np.zeros((B, TILES, KW), dtype=np.float32)
    gt_w = np.zeros((B, TILES, KW, KW), dtype=np.float16)
    oi = np.arange(O)
    for b in range(B):
        for t in range(TILES):
            off, v2 = _tile_off(t), _tile_v2(t)
            sel = np.nonzero((ids[b] >= off) & (ids[b] < off + GPT * v2))[0]
            c = sel.size
            if c > KW:
                raise RuntimeError(
                    f"id window overflow: batch {b} span {t} has {c} > {KW} ids"
                )
            rel = ids[b, sel] - off
            hi = rel // v2
            lo_w[b, t, :c] = rel % v2
            # gt[i, o*4+g] = attn[b, o, sel[i]] * (hi[i] == g)
            cols = attn[b][:, sel].T.astype(np.float16)  # [c, O]
            gt_w[b, t, np.arange(c)[:, None], oi[None, :] * GPT + hi[:, None]] = cols
    lo_t = lo_w.reshape(NCORES, NW, KW).transpose(0, 2, 1)  # [8, 128, NW]
    gt_t = gt_w.reshape(NCORES, BPC, TILES, KW, KW).transpose(
        0, 1, 3, 2, 4).reshape(NCORES, BPC, KW, TILES * KW)
    lov = np.broadcast_to(
        np.arange(V2, dtype=np.float16)[None, :], (128, V2))
    in_maps = [
        {
            "gtj": np.ascontiguousarray(gt_t[c]),
            "lof": np.ascontiguousarray(lo_t[c]),
            "lov": np.ascontiguousarray(lov),
        }
        for c in range(NCORES)
    ]
    return in_maps


def kernel(ids, attn):
    from concourse.bass_utils import run_bass_kernel_spmd

    ids = np.ascontiguousarray(ids, dtype=np.int32)
    attn = np.ascontiguousarray(attn, dtype=np.float32)

    if "nc" not in _cache:
        _cache["nc"] = _build()
    nc = _cache["nc"]

    core_ids = list(range(NCORES))
    res = run_bass_kernel_spmd(nc, _in_maps(ids, attn), core_ids)
    out = np.concatenate([res.results[c]["out"] for c in core_ids], axis=0)
    return out


# revision 12
# speedup vs baseline: 9.1654x; 1.1751x over previous
"""Trainium2 Bass kernel for batched weighted scatter-add (AttentionCopy).

Computes out[b, o, v] = sum_i attn[b, o, i] * (ids[b, i] == v)
for ids [16, 512] int32 in [0, 50000), attn [16, 32, 512] f32,
out [16, 32, 50000] f32.

Pure data parallel over batch: 2 batches per core on 8 cores. Per batch the
[32, 50000] output is built densely as 7 SBUF tiles: 6 of [128, 2048] f32
(vocab span 8192 = 4 groups x 2048; seed-0 data puts at most 98 of 512 ids
in any such span, capacity 128) plus a [128, 212] tail. Tile rows are
(o, g) pairs (p = o*4 + g, g = rel // 2048), columns lo = rel % 2048, so
each partition's 8 KB row is contiguous in DRAM: descriptors are 8 KB
(vs 2.5 KB in the v1 4096-span kernel; measured per-SDMA-engine rate
improves ~24 -> ~25.5 GB/s) and there are 4x fewer of them.

Each output DMA is a [32, 4, 2048] access pattern. The HWDGE sprays
descriptors over the 16 SDMA engines BY OUTER-DIM INDEX (engine = o mod
16) - NOT by SBUF partition - and only outer counts that are <= 16 or
divisible by 16 spray at all (a [23, 4, 2048] AP lands entirely on engine
0, measured). So outer dim stays 32 and each engine moves one [4, 2048]
block for its two o values.

Per tile the device does a one-hot matmul pass: alo[i, c] = (lo_i == c)
(single vector op per tile; fp16 holds integers exactly up to 2048), then
out[(o,g), c] = gt.T @ alo with gt[i, (o,g)] = attn[b, o, i] masked by
(g == hi_i), packed on host. The compare constant lov[p, c] = c is a DMA
input (the v1 gpsimd iota took 1.8 us and serialized the first build; the
gpsimd tensor_scalar path is a ~40x-slow emulation - measured - so all
builds stay on the vector engine).

The PE HAM clock gate holds the tensor engine at 1.2 GHz unless it is
nearly always busy (v1: 630 ns matmuls vs the 216 ns warm rate). Dummy
"warmer" matmuls into a scratch PSUM bank pad each tile's PE work so the
array stays at K=8/8.

Steady-state per-tile engine budget (target cadence ~2.6 us = 1 MB tile /
16 engines / 25.5 GB/s): vector = alo build 0.6 + psB copy 1.2; scalar =
psA copy 1.0 + every-other-tile DMA kick 0.45; tensor = 4 matmuls + 2
warmers ~1.9 warm; sync = other DMA kicks.
"""

import sys

sys.path.insert(0, "/opt/trn_rl_repo")

import numpy as np

NCORES = 8
B, O, I = 16, 32, 512
SIZE = 50000
BPC = B // NCORES  # batches per core
V2 = 2048  # per-partition columns per full tile (fp16 int-exact limit)
V2T = 212  # tail tile: 6*4*2048 + 4*212 = 50000
GPT = 4  # groups per tile: 128 rows = 32 o x 4 groups
SPAN = GPT * V2  # 8192 vocab per full tile
TILES = 7  # 6 full + 1 tail per batch
KW = 128  # id-window capacity per (batch, tile)
NW = BPC * TILES  # 14 windows per core
LA = 2  # alo build lookahead (windows)
NWARM = 0  # PE warmer matmuls per full window (cold PE fits the cadence)

_cache = {}


def _tile_v2(t):
    return V2 if t < TILES - 1 else V2T


def _tile_off(t):
    return t * SPAN  # tail starts at 6*8192 = 49152


def _build():
    import concourse.bacc as bacc
    import concourse.mybir as mybir
    import concourse.tile as tile

    f32 = mybir.dt.float32
    f16 = mybir.dt.float16
    Alu = mybir.AluOpType

    nc = bacc.Bacc("TRN2", target_bir_lowering=False, debug=False, num_devices=NCORES)

    # host-packed stationary matrices: [b, i_slot, t*KW + (o*4+g)]
    gt_d = nc.dram_tensor("gtj", [BPC, 128, TILES * KW], f16, kind="ExternalInput").ap()
    # lo per window slot: [p, b*TILES+t] (0 pad)
    lof_d = nc.dram_tensor("lof", [128, NW], f32, kind="ExternalInput").ap()
    # iota constant lov[p, c] = c, first half; [1024:2048] built on device
    lov_d = nc.dram_tensor("lov", [128, 1024], f16, kind="ExternalInput").ap()
    out_d = nc.dram_tensor("out", [BPC, O, SIZE], f32, kind="ExternalOutput").ap()

    with tile.TileContext(nc) as tc:
        with (
            tc.tile_pool(name="const", bufs=1) as constp,
            tc.tile_pool(name="alo", bufs=LA + 2) as alop,
            # deep outs pool: SDMA engine 15 runs ~20% slower than 0-14
            # (persistent trn2 quirk); letting its completions lag ~8 windows
            # keeps the other 15 engines at their own pace instead of
            # throttling every window to engine 15's cadence
            tc.tile_pool(name="outs", bufs=9) as outp,
            tc.tile_pool(name="psmm", bufs=4, space="PSUM") as psmm,
        ):
            # --- inputs: iota constant first (gates the first alo build),
            # then tail gt blocks (first windows), then the rest
            lov = constp.tile([128, V2], f16, tag="lov")
            nc.sync.dma_start(out=lov[:, 0:512], in_=lov_d[:, 0:512])
            nc.scalar.dma_start(out=lov[:, 512:1024], in_=lov_d[:, 512:1024])
            nc.vector.tensor_scalar_add(out=lov[:, 1024:2048],
                                        in0=lov[:, 0:1024], scalar1=1024.0)

            lo_f = constp.tile([128, NW], f32, tag="lo_f")
            nc.scalar.dma_start(out=lo_f[:], in_=lof_d[:])

            TKW = TILES * KW
            tb0, tb1 = (TILES - 1) * KW, TILES * KW  # tail gt column block
            gts = []
            for b in range(BPC):
                t_ = constp.tile([128, TKW], f16, tag=f"gt{b}", name=f"gt{b}")
                nc.sync.dma_start(out=t_[:, tb0:tb1], in_=gt_d[b][:, tb0:tb1])
                nc.sync.dma_start(out=t_[:, 0:384], in_=gt_d[b][:, 0:384])
                nc.scalar.dma_start(out=t_[:, 384:tb0], in_=gt_d[b][:, 384:tb0])
                gts.append(t_)

            # window order: tails lead each batch (their small-descriptor DMA
            # hides in the ramp instead of serializing the drain)
            order = [(0, TILES - 1)] + [(0, t) for t in range(TILES - 1)]
            order += [(1, TILES - 1)] + [(1, t) for t in range(TILES - 1)]

            alos = {}

            def build(w):
                b, t = w
                v2 = _tile_v2(t)
                wi = b * TILES + t
                alo = alop.tile([128, V2], f16, tag="alo", name=f"alo{b}_{t}")
                nc.vector.tensor_scalar(
                    out=alo[:, 0:v2], in0=lov[:, 0:v2],
                    scalar1=lo_f[:, wi : wi + 1],
                    scalar2=None, op0=Alu.is_equal)
                alos[w] = alo

            for k in range(LA):
                build(order[k])

            for k in range(len(order)):
                if k + LA < len(order):
                    build(order[k + LA])
                b, t = w = order[k]
                v2 = _tile_v2(t)
                alo = alos[w]
                gt = gts[b][:, t * KW : (t + 1) * KW]
                qa = nc.sync if k % 2 == 0 else nc.scalar

                off = _tile_off(t)
                outv = out_d[b][:, off : off + GPT * v2].rearrange(
                    "o (g l) -> o g l", l=v2)

                if t == TILES - 1:
                    # tail: one matmul, scalar copy, single DMA
                    ps = psmm.tile([128, 1024], f32, tag="mm", name=f"ps{b}t")
                    nc.tensor.matmul(out=ps[:, 0:V2T], lhsT=gt,
                                     rhs=alo[:, 0:V2T], start=True, stop=True)
                    os_ = outp.tile([128, V2], f32, tag="os", name=f"os{b}t")
                    nc.scalar.copy(out=os_[:, 0:V2T], in_=ps[:, 0:V2T])
                    qa.dma_start(out=outv, in_=os_[:, 0:V2T])
                    continue

                psA = psmm.tile([128, 1024], f32, tag="mm", name=f"psA{b}_{t}")
                psB = psmm.tile([128, 1024], f32, tag="mm", name=f"psB{b}_{t}")
                nc.tensor.matmul(out=psA[:, 0:512], lhsT=gt,
                                 rhs=alo[:, 0:512], start=True, stop=True)
                nc.tensor.matmul(out=psA[:, 512:1024], lhsT=gt,
                                 rhs=alo[:, 512:1024], start=True, stop=True)
                nc.tensor.matmul(out=psB[:, 0:512], lhsT=gt,
                                 rhs=alo[:, 1024:1536], start=True, stop=True)
                nc.tensor.matmul(out=psB[:, 512:1024], lhsT=gt,
                                 rhs=alo[:, 1536:2048], start=True, stop=True)

                os_ = outp.tile([128, V2], f32, tag="os", name=f"os{b}_{t}")
                nc.scalar.copy(out=os_[:, 0:1024], in_=psA[:, 0:1024])
                nc.vector.tensor_copy(out=os_[:, 1024:2048],
                                      in_=psB[:, 0:1024])

                if k == 1:
                    # first full window: kick each column half as soon as its
                    # copy lands to start the output stream earlier
                    qa.dma_start(out=outv[:, :, 0:1024],
                                 in_=os_[:, 0:1024])
                    qa.dma_start(out=outv[:, :, 1024:2048],
                                 in_=os_[:, 1024:2048])
                else:
                    qa.dma_start(out=outv, in_=os_[:, 0:2048])

    nc.compile()
    return nc


def _in_maps(ids, attn):
    lo_w = np.zeros((B, TILES, KW), dtype=np.float32)
    gt_w = np.zeros((B, TILES, KW, KW), dtype=np.float16)
    oi = np.arange(O)
    for b in range(B):
        for t in range(TILES):
            off, v2 = _tile_off(t), _tile_v2(t)
            sel = np.nonzero((ids[b] >= off) & (ids[b] < off + GPT * v2))[0]
            c = sel.size
            if c > KW:
                raise RuntimeError(
                    f"id window overflow: batch {b} span {t} has {c} > {KW} ids"
                )
            rel = ids[b, sel] - off
            hi = rel // v2
            lo_w[b, t, :c] = rel % v2
            # gt[i, o*4+g] = attn[b, o, sel[i]] * (hi[i] == g)
            cols = attn[b][:, sel].T.astype(np.float16)  # [c, O]
            gt_w[b, t, np.arange(c)[:, None], oi[None, :] * GPT + hi[:, None]] = cols
    lo_t = lo_w.reshape(NCORES, NW, KW).transpose(0, 2, 1)  # [8, 128, NW]
    gt_t = gt_w.reshape(NCORES, BPC, TILES, KW, KW).transpose(
        0, 1, 3, 2, 4).reshape(NCORES, BPC, KW, TILES * KW)
    lov = np.broadcast_to(
        np.arange(1024, dtype=np.float16)[None, :], (128, 1024))
    in_maps = [
        {
            "gtj": np.ascontiguousarray(gt_t[c]),
            "lof": np.ascontiguousarray(lo_t[c]),
            "lov": np.ascontiguousarray(lov),
        }
        for c in range(NCORES)
    ]
    return in_maps


def kernel(ids, attn):
    from concourse.bass_utils import run_bass_kernel_spmd

    ids = np.ascontiguousarray(ids, dtype=np.int32)
    attn = np.ascontiguousarray(attn, dtype=np.float32)

    if "nc" not in _cache:
        _cache["nc"] = _build()
    nc = _cache["nc"]

    core_ids = list(range(NCORES))
    res = run_bass_kernel_spmd(nc, _in_maps(ids, attn), core_ids)
    out = np.concatenate([res.results[c]["out"] for c in core_ids], axis=0)
    return out
